# revision 12
# baseline (speedup 1.0000x reference)
"""Trainium2 Bass kernel for nn_Attention_4080218931831 (sparse_attention).

Computes, for each batch b:
    q = s_b @ Qw           [512, 32]
    k = s_b @ Kw           [512, 32]
    scores = q @ k^T       [512, 512]
    att = scores^2 * G_b
    out = att / (sum(att, axis=-1, keepdims=True) + 0.001)

Algebraic refactor: scores = s_b @ (Qw @ Kw^T) @ s_b^T = s_b @ t_b where
t_b = A @ s_b^T and A = Qw @ Kw^T is [10, 10].  A and t are precomputed on
the host in float64 (0.06% of total FLOPs); the dominant [512,10]x[10,512]
matmul per batch runs on the PE.

PE precision strategy: fp32 matmul on trn2 costs 4 cycles/row (two
half-speed passes).  Instead we split both operands into bf16 hi+lo
(s = sh + sl, t = th + tl) and compute scores = sh.th + [sh;sl].[tl;th]
(two 1-cycle/row bf16 matmuls accumulated in fp32 PSUM, contraction 10 and
20).  Only the sl.tl term is dropped (~2^-18 relative) giving ~1e-5
end-to-end relative error at half the fp32 PE cost.

Per-core pipeline per batch (32 batches/core, 4 row-chunks of 128):
  PE:  scores chunk -> PSUM [128,512]
  ACT: sq = Square(scores)  PSUM->SBUF
  DVE: scalar_tensor_tensor: att = sq*G, den_col = rowsum(att)
  DVE: rec = 1/(den + 0.001)  per batch
  DVE/ACT (split): out_chunk = att * rec[:, c]
  1 MiB DMAs for G in / out.

Sharding: pure data parallel - batch axis 256 split as 32 per core over 8
cores.  Weights are folded into t on the host.
"""

import numpy as np

# Problem shapes (hardcoded per contract)
B_FULL = 256
N = 512
K_IN = 10
HID = 32
N_CORES = 8
B_LOC = B_FULL // N_CORES  # 32
P = 128                    # SBUF partitions per row-chunk
N_CHUNK = N // P           # 4

# How many of the 4 per-batch final-scale chunks run on ACT (rest on DVE)
ACT_SCALE_CHUNKS = 1

_cache = {}


def _build_nc(b_loc=B_LOC):
    import concourse.mybir as mybir
    from concourse import bacc
    from concourse.tile import TileContext
    from contextlib import ExitStack

    f32 = mybir.dt.float32
    bf16 = mybir.dt.bfloat16
    nc = bacc.Bacc("TRN2", target_bir_lowering=False, debug=False,
                   num_devices=N_CORES)

    # One K=30 bf16 matmul per chunk: lhs = [sh;sh;sl], rhs = [th;tl;th]
    lhs_d = nc.dram_tensor("lhs", [b_loc, 3 * K_IN, N], bf16,
                           kind="ExternalInput")
    rhs_d = nc.dram_tensor("rhs", [b_loc, 3 * K_IN, N], bf16,
                           kind="ExternalInput")
    G_d = nc.dram_tensor("G", [b_loc, N, N], f32, kind="ExternalInput")
    out_d = nc.dram_tensor("out", [b_loc, N, N], f32, kind="ExternalOutput")

    # DMA batching: fixed ~750ns issue cost per dma_start on the Sync queue
    # dominates at 1 MiB grain, so G/out move 2 batches (2 MiB) per DMA and
    # the small bf16 operands 4 batches per DMA.
    GB = 2        # batches per G/out DMA
    SB = 4        # batches per lhs/rhs DMA

    with TileContext(nc) as tc, ExitStack() as ctx:
        st_pool = ctx.enter_context(tc.tile_pool(name="st", bufs=2))
        g_pool = ctx.enter_context(tc.tile_pool(name="g", bufs=4))
        sq_pool = ctx.enter_context(tc.tile_pool(name="sq", bufs=6))
        att_pool = ctx.enter_context(tc.tile_pool(name="att", bufs=4))
        out_pool = ctx.enter_context(tc.tile_pool(name="o", bufs=3))
        den_pool = ctx.enter_context(tc.tile_pool(name="den", bufs=3))
        ps_pool = ctx.enter_context(tc.tile_pool(name="ps", bufs=6, space="PSUM"))

        st_tiles = {}
        for bb in range(0, b_loc, GB):
            if bb % SB == 0:
                lhs_t = st_pool.tile([3 * K_IN, SB, N], bf16, tag="lhs")
                rhs_t = st_pool.tile([3 * K_IN, SB, N], bf16, tag="rhs")
                nc.sync.dma_start(
                    out=lhs_t,
                    in_=lhs_d.ap()[bb:bb + SB].rearrange("b k n -> k b n"))
                nc.sync.dma_start(
                    out=rhs_t,
                    in_=rhs_d.ap()[bb:bb + SB].rearrange("b k n -> k b n"))
                st_tiles = {"lhs": lhs_t, "rhs": rhs_t}

            # 2 batches of G in one 2 MiB DMA.  Interleaved row layout:
            # attention row n = 4p + j lives at partition p, free-slot j, so
            # every partition's slice of G_b is 8 KiB contiguous in HBM.
            g_t = g_pool.tile([P, GB, N_CHUNK, N], f32, tag="G")
            nc.sync.dma_start(
                out=g_t,
                in_=G_d.ap()[bb:bb + GB].rearrange("b (p j) n -> p b j n", p=P))

            o_t = out_pool.tile([P, GB, N_CHUNK, N], f32, tag="o")

            for i in range(GB):
                b = bb + i
                si = b % SB
                # lhsT view: chunk j selects columns n = 4p + j (stride 4)
                # of the [20, 512] stationary operand for this batch.
                lhs_v = st_tiles["lhs"][:, si, :].rearrange(
                    "k (p j) -> k j p", j=N_CHUNK)
                rhs_b = st_tiles["rhs"][:, si, :]

                att_t = att_pool.tile([P, N_CHUNK, N], f32, tag="att")
                den_t = den_pool.tile([P, N_CHUNK], f32, tag="den")
                rec_t = den_pool.tile([P, N_CHUNK], f32, tag="rec")

                for c in range(N_CHUNK):
                    ps = ps_pool.tile([P, N], f32, tag="ps")
                    # scores = sh.th + sh.tl + sl.th (one K=30 bf16 matmul)
                    nc.tensor.matmul(
                        out=ps,
                        lhsT=lhs_v[:, c, :],
                        rhs=rhs_b,
                        start=True, stop=True,
                    )
                    sq = sq_pool.tile([P, N], f32, tag="sq")
                    nc.scalar.activation(
                        out=sq, in_=ps,
                        func=mybir.ActivationFunctionType.Square)
                    # att = sq * G ; den = sum(att, axis=-1)
                    nc.vector.scalar_tensor_tensor(
                        out=att_t[:, c, :],
                        in0=sq,
                        scalar=1.0,
                        in1=g_t[:, i, c, :],
                        op0=mybir.AluOpType.mult,
                        op1=mybir.AluOpType.mult,
                        accum_out=den_t[:, c:c + 1],
                    )

                # rec = 1 / (den + 0.001)
                nc.vector.tensor_scalar_add(
                    out=rec_t, in0=den_t, scalar1=0.001)
                nc.vector.reciprocal(out=rec_t, in_=rec_t)

                # Alternate 2/1 ACT scale chunks per batch (avg 1.5) to
                # balance ACT (~710ns/op) vs DVE (~400ns/op) totals.
                n_act = ACT_SCALE_CHUNKS + (b % 2)
                for c in range(N_CHUNK):
                    if c < n_act:
                        nc.scalar.mul(o_t[:, i, c, :], att_t[:, c, :],
                                      rec_t[:, c:c + 1])
                    else:
                        nc.vector.tensor_scalar_mul(
                            o_t[:, i, c, :], att_t[:, c, :],
                            rec_t[:, c:c + 1])

            # Output DMA issues from the ACT HWDGE ring so a not-yet-ready
            # output can never head-of-line-block the next G input issue on
            # the Sync ring.
            nc.scalar.dma_start(
                out=out_d.ap()[bb:bb + GB].rearrange(
                    "b (p j) n -> p b j n", p=P),
                in_=o_t)

    nc.compile()
    return nc


def _host_prep(s, Qweight, Kweight):
    """Returns bf16 hi/lo packed lhs [B,30,N] = [sh;sh;sl] and
    rhs [B,30,N] = [th;tl;th] so one K=30 bf16 matmul computes
    sh.th + sh.tl + sl.th."""
    import ml_dtypes
    bf = ml_dtypes.bfloat16
    s = np.asarray(s, dtype=np.float32)
    A = np.asarray(Qweight, np.float64) @ np.asarray(Kweight, np.float64).T
    sT = np.ascontiguousarray(s.transpose(0, 2, 1))          # [B, 10, N]
    t = np.einsum("kl,bln->bkn", A, sT.astype(np.float64)).astype(np.float32)

    sh = sT.astype(bf)
    sl = (sT - sh.astype(np.float32)).astype(bf)
    th = t.astype(bf)
    tl = (t - th.astype(np.float32)).astype(bf)

    lhs = np.concatenate([sh, sh, sl], axis=1)   # [B, 30, N]
    rhs = np.concatenate([th, tl, th], axis=1)   # [B, 30, N]
    return np.ascontiguousarray(lhs), np.ascontiguousarray(rhs)


def _run(in_maps, trace=False, **kw):
    from concourse.bass_utils import run_bass_kernel_spmd
    if "nc" not in _cache:
        _cache["nc"] = _build_nc()
    nc = _cache["nc"]
    return run_bass_kernel_spmd(
        nc, in_maps, core_ids=list(range(N_CORES)), trace=trace, **kw)


def _make_in_maps(s, Gmat, Qweight, Kweight):
    lhs, rhs = _host_prep(s, Qweight, Kweight)
    Gmat = np.asarray(Gmat, dtype=np.float32)
    in_maps = []
    for c in range(N_CORES):
        sl = slice(c * B_LOC, (c + 1) * B_LOC)
        in_maps.append({
            "lhs": np.ascontiguousarray(lhs[sl]),
            "rhs": np.ascontiguousarray(rhs[sl]),
            "G": np.ascontiguousarray(Gmat[sl]),
        })
    return in_maps


def kernel_traced(s, Gmat, Qweight, Kweight, trace=True):
    """Like kernel() but returns (output, BassKernelResults)."""
    in_maps = _make_in_maps(s, Gmat, Qweight, Kweight)
    res = _run(in_maps, trace=trace)
    out = np.concatenate([r["out"] for r in res.results], axis=0)
    return out, res


def kernel(s, Gmat, Qweight, Kweight):
    out, _ = kernel_traced(s, Gmat, Qweight, Kweight, trace=False)
    return out


# revision 13
# speedup vs baseline: 1.0080x; 1.0080x over previous
"""Trainium2 Bass kernel for nn_Attention_4080218931831 (sparse_attention).

Computes, for each batch b:
    q = s_b @ Qw           [512, 32]
    k = s_b @ Kw           [512, 32]
    scores = q @ k^T       [512, 512]
    att = scores^2 * G_b
    out = att / (sum(att, axis=-1, keepdims=True) + 0.001)

Algebraic refactor: scores = s_b @ (Qw @ Kw^T) @ s_b^T = s_b @ t_b where
t_b = A @ s_b^T and A = Qw @ Kw^T is [10, 10].  A and t are precomputed on
the host in float64 (0.06% of total FLOPs); the dominant [512,10]x[10,512]
matmul per batch runs on the PE.

PE precision strategy: fp32 matmul on trn2 costs 4 cycles/row (two
half-speed passes).  Instead we split both operands into bf16 hi+lo
(s = sh + sl, t = th + tl) and compute scores = sh.th + [sh;sl].[tl;th]
(two 1-cycle/row bf16 matmuls accumulated in fp32 PSUM, contraction 10 and
20).  Only the sl.tl term is dropped (~2^-18 relative) giving ~1e-5
end-to-end relative error at half the fp32 PE cost.

Per-core pipeline per batch (32 batches/core, 4 row-chunks of 128):
  PE:  scores chunk -> PSUM [128,512]
  ACT: sq = Square(scores)  PSUM->SBUF
  DVE: scalar_tensor_tensor: att = sq*G, den_col = rowsum(att)
  DVE: rec = 1/(den + 0.001)  per batch
  DVE/ACT (split): out_chunk = att * rec[:, c]
  1 MiB DMAs for G in / out.

Sharding: pure data parallel - batch axis 256 split as 32 per core over 8
cores.  Weights are folded into t on the host.
"""

import numpy as np

# Problem shapes (hardcoded per contract)
B_FULL = 256
N = 512
K_IN = 10
HID = 32
N_CORES = 8
B_LOC = B_FULL // N_CORES  # 32
P = 128                    # SBUF partitions per row-chunk
N_CHUNK = N // P           # 4

# How many of the 4 per-batch final-scale chunks run on ACT (rest on DVE)
ACT_SCALE_CHUNKS = 1

_cache = {}


def _build_nc(b_loc=B_LOC):
    import concourse.mybir as mybir
    from concourse import bacc
    from concourse.tile import TileContext
    from contextlib import ExitStack

    f32 = mybir.dt.float32
    bf16 = mybir.dt.bfloat16
    nc = bacc.Bacc("TRN2", target_bir_lowering=False, debug=False,
                   num_devices=N_CORES)

    # One K=30 bf16 matmul per chunk: lhs = [sh;sh;sl], rhs = [th;tl;th]
    lhs_d = nc.dram_tensor("lhs", [b_loc, 3 * K_IN, N], bf16,
                           kind="ExternalInput")
    rhs_d = nc.dram_tensor("rhs", [b_loc, 3 * K_IN, N], bf16,
                           kind="ExternalInput")
    G_d = nc.dram_tensor("G", [b_loc, N, N], f32, kind="ExternalInput")
    out_d = nc.dram_tensor("out", [b_loc, N, N], f32, kind="ExternalOutput")

    # DMA batching: fixed ~750ns issue cost per dma_start on the Sync queue
    # dominates at 1 MiB grain, so G/out move 2 batches (2 MiB) per DMA and
    # the small bf16 operands 4 batches per DMA.
    GB = 1        # batches per G/out DMA
    SB = 4        # batches per lhs/rhs DMA

    with TileContext(nc) as tc, ExitStack() as ctx:
        st_pool = ctx.enter_context(tc.tile_pool(name="st", bufs=2))
        g_pool = ctx.enter_context(tc.tile_pool(name="g", bufs=4))
        sq_pool = ctx.enter_context(tc.tile_pool(name="sq", bufs=6))
        att_pool = ctx.enter_context(tc.tile_pool(name="att", bufs=4))
        out_pool = ctx.enter_context(tc.tile_pool(name="o", bufs=3))
        den_pool = ctx.enter_context(tc.tile_pool(name="den", bufs=3))
        ps_pool = ctx.enter_context(tc.tile_pool(name="ps", bufs=6, space="PSUM"))

        st_tiles = {}
        for bb in range(0, b_loc, GB):
            if bb % SB == 0:
                lhs_t = st_pool.tile([3 * K_IN, SB, N], bf16, tag="lhs")
                rhs_t = st_pool.tile([3 * K_IN, SB, N], bf16, tag="rhs")
                nc.sync.dma_start(
                    out=lhs_t,
                    in_=lhs_d.ap()[bb:bb + SB].rearrange("b k n -> k b n"))
                nc.sync.dma_start(
                    out=rhs_t,
                    in_=rhs_d.ap()[bb:bb + SB].rearrange("b k n -> k b n"))
                st_tiles = {"lhs": lhs_t, "rhs": rhs_t}

            # 2 batches of G in one 2 MiB DMA.  Interleaved row layout:
            # attention row n = 4p + j lives at partition p, free-slot j, so
            # every partition's slice of G_b is 8 KiB contiguous in HBM.
            g_t = g_pool.tile([P, GB, N_CHUNK, N], f32, tag="G")
            nc.sync.dma_start(
                out=g_t,
                in_=G_d.ap()[bb:bb + GB].rearrange("b (p j) n -> p b j n", p=P))

            o_t = out_pool.tile([P, GB, N_CHUNK, N], f32, tag="o")

            for i in range(GB):
                b = bb + i
                si = b % SB
                # lhsT view: chunk j selects columns n = 4p + j (stride 4)
                # of the [20, 512] stationary operand for this batch.
                lhs_v = st_tiles["lhs"][:, si, :].rearrange(
                    "k (p j) -> k j p", j=N_CHUNK)
                rhs_b = st_tiles["rhs"][:, si, :]

                att_t = att_pool.tile([P, N_CHUNK, N], f32, tag="att")
                den_t = den_pool.tile([P, N_CHUNK], f32, tag="den")
                rec_t = den_pool.tile([P, N_CHUNK], f32, tag="rec")

                for c in range(N_CHUNK):
                    ps = ps_pool.tile([P, N], f32, tag="ps")
                    # scores = sh.th + sh.tl + sl.th (one K=30 bf16 matmul)
                    nc.tensor.matmul(
                        out=ps,
                        lhsT=lhs_v[:, c, :],
                        rhs=rhs_b,
                        start=True, stop=True,
                    )
                    sq = sq_pool.tile([P, N], f32, tag="sq")
                    nc.scalar.activation(
                        out=sq, in_=ps,
                        func=mybir.ActivationFunctionType.Square)
                    # att = sq * G ; den = sum(att, axis=-1)
                    nc.vector.scalar_tensor_tensor(
                        out=att_t[:, c, :],
                        in0=sq,
                        scalar=1.0,
                        in1=g_t[:, i, c, :],
                        op0=mybir.AluOpType.mult,
                        op1=mybir.AluOpType.mult,
                        accum_out=den_t[:, c:c + 1],
                    )

                # rec = 1 / (den + 0.001)
                nc.vector.tensor_scalar_add(
                    out=rec_t, in0=den_t, scalar1=0.001)
                nc.vector.reciprocal(out=rec_t, in_=rec_t)

                # Alternate 2/1 ACT scale chunks per batch (avg 1.5) to
                # balance ACT (~710ns/op) vs DVE (~400ns/op) totals.
                n_act = ACT_SCALE_CHUNKS + (b % 2)
                for c in range(N_CHUNK):
                    if c < n_act:
                        nc.scalar.mul(o_t[:, i, c, :], att_t[:, c, :],
                                      rec_t[:, c:c + 1])
                    else:
                        nc.vector.tensor_scalar_mul(
                            o_t[:, i, c, :], att_t[:, c, :],
                            rec_t[:, c:c + 1])

            # Output DMA issues from the ACT HWDGE ring so a not-yet-ready
            # output can never head-of-line-block the next G input issue on
            # the Sync ring.
            nc.scalar.dma_start(
                out=out_d.ap()[bb:bb + GB].rearrange(
                    "b (p j) n -> p b j n", p=P),
                in_=o_t)

    nc.compile()
    return nc


def _host_prep(s, Qweight, Kweight):
    """Returns bf16 hi/lo packed lhs [B,30,N] = [sh;sh;sl] and
    rhs [B,30,N] = [th;tl;th] so one K=30 bf16 matmul computes
    sh.th + sh.tl + sl.th."""
    import ml_dtypes
    bf = ml_dtypes.bfloat16
    s = np.asarray(s, dtype=np.float32)
    A = np.asarray(Qweight, np.float64) @ np.asarray(Kweight, np.float64).T
    sT = np.ascontiguousarray(s.transpose(0, 2, 1))          # [B, 10, N]
    t = np.einsum("kl,bln->bkn", A, sT.astype(np.float64)).astype(np.float32)

    sh = sT.astype(bf)
    sl = (sT - sh.astype(np.float32)).astype(bf)
    th = t.astype(bf)
    tl = (t - th.astype(np.float32)).astype(bf)

    lhs = np.concatenate([sh, sh, sl], axis=1)   # [B, 30, N]
    rhs = np.concatenate([th, tl, th], axis=1)   # [B, 30, N]
    return np.ascontiguousarray(lhs), np.ascontiguousarray(rhs)


def _run(in_maps, trace=False, **kw):
    from concourse.bass_utils import run_bass_kernel_spmd
    if "nc" not in _cache:
        _cache["nc"] = _build_nc()
    nc = _cache["nc"]
    return run_bass_kernel_spmd(
        nc, in_maps, core_ids=list(range(N_CORES)), trace=trace, **kw)


def _make_in_maps(s, Gmat, Qweight, Kweight):
    lhs, rhs = _host_prep(s, Qweight, Kweight)
    Gmat = np.asarray(Gmat, dtype=np.float32)
    in_maps = []
    for c in range(N_CORES):
        sl = slice(c * B_LOC, (c + 1) * B_LOC)
        in_maps.append({
            "lhs": np.ascontiguousarray(lhs[sl]),
            "rhs": np.ascontiguousarray(rhs[sl]),
            "G": np.ascontiguousarray(Gmat[sl]),
        })
    return in_maps


def kernel_traced(s, Gmat, Qweight, Kweight, trace=True):
    """Like kernel() but returns (output, BassKernelResults)."""
    in_maps = _make_in_maps(s, Gmat, Qweight, Kweight)
    res = _run(in_maps, trace=trace)
    out = np.concatenate([r["out"] for r in res.results], axis=0)
    return out, res


def kernel(s, Gmat, Qweight, Kweight):
    out, _ = kernel_traced(s, Gmat, Qweight, Kweight, trace=False)
    return out
